# revision 1
# baseline (speedup 1.0000x reference)
"""Trainium2 Bass kernel for the forward-attention LSA step (nn_LSA_43404939494068).

Contract: kernel(**inputs) takes the FULL inputs from setup_inputs() and
returns the FULL output [64, 1, 1024] float32. Internally shards batch
across 8 NeuronCores (8 batches each), runs one Bass/Tile program SPMD.

Math notes (vs reference):
  u[b,t]   = sum_a v[a] * tanh(pq[b,a] + enc[b,t,a] + ploc[b,t,a])
  ploc     = conv1d([cumulative; attention]) @ L_w.T + L_b; the conv and the
             L-projection fold into ONE matmul: ploc[t,:] = ls[:,t].T @ M,
             M[j,a] = sum_f conv_w[f,c_j,k_j] * L_w[a,f] (host-precomputed
             weight algebra), ls = 62 shifted copies of the two loc rows
             (+ a ones row that carries pq+W_b+L_b into the same matmul).
  The reference's division of s=sigmoid(u) by sum(s) cancels exactly in the
  final alpha normalization, so it is skipped.

Per [128t x 512a] tile: PE matmul (K=63: folded conv+proj+bias) and PE
identity-matmul (accumulates enc into the same PSUM bank), ACT tanh, DVE
tensor_tensor_reduce (x*v and row-sum fused) -> one u column. The tail
(sigmoid, mask, alpha-shift recursion, normalize) runs in a [t',(k,b)]
64-column layout with the t-shifts expressed as two small band-matrix
matmuls, then one PE transpose and a single output DMA.
"""

import sys

import numpy as np

if "/opt/trn_rl_repo" not in sys.path:
    sys.path.insert(0, "/opt/trn_rl_repo")

import concourse.bass as bass
import concourse.tile as tile
from concourse import mybir
from concourse.bass_utils import run_bass_kernel_spmd

B, T, A = 64, 1024, 512
F, KW = 32, 31
PAD = (KW - 1) // 2
NCORES = 8
LB = B // NCORES          # 8 local batches per core
NK = T // 128             # 8 t-tiles of 128
KC = 62                   # conv contraction = 2 channels * 31 taps
KP = 640                  # 513 padded to 5*128 for the pq matmul
F32 = mybir.dt.float32

_MAX_WAITS = 1


def _split_sync_waits(nc):
    """walrus in this toolchain accepts at most one sync-wait per
    instruction; hoist excess waits onto NoOps inserted just before."""
    for fn in nc.m.functions:
        for blk in fn.blocks:
            new_list = []
            for inst in blk.instructions:
                si = inst.sync_info
                if si is not None and si.on_wait and len(si.on_wait) > _MAX_WAITS:
                    waits = list(si.on_wait)
                    extra, keep = waits[:-_MAX_WAITS], waits[-_MAX_WAITS:]
                    for i in range(0, len(extra), _MAX_WAITS):
                        nop = mybir.InstNoOp(
                            name=nc.get_next_instruction_name(),
                            sync_info=mybir.SyncInfo(
                                on_wait=extra[i:i + _MAX_WAITS], on_update=[]
                            ),
                            bass_nofuse=True,
                            engine=inst.engine,
                        )
                        nc.register_instruction(nop)
                        new_list.append(nop)
                    inst.sync_info = mybir.SyncInfo(
                        on_wait=keep, on_update=list(si.on_update)
                    )
                new_list.append(inst)
            blk.instructions[:] = new_list


def build_program(repeats: int = 1) -> bass.Bass:
    nc = bass.Bass()
    dt = F32

    enc_d = nc.declare_dram_parameter("enc", [LB, T, A], dt, isOutput=False)
    ls_d = nc.declare_dram_parameter("ls", [LB, KC + 1, T], dt, isOutput=False)
    qt_d = nc.declare_dram_parameter("qt", [KP, LB], dt, isOutput=False)
    qw_d = nc.declare_dram_parameter("qw", [KP, A], dt, isOutput=False)
    mcomb_d = nc.declare_dram_parameter("mcomb", [KC, A], dt, isOutput=False)
    vw_d = nc.declare_dram_parameter("vw", [A], dt, isOutput=False)
    eye_d = nc.declare_dram_parameter("eye", [128, 128], dt, isOutput=False)
    ones_d = nc.declare_dram_parameter("ones", [128, 128], dt, isOutput=False)
    tri_d = nc.declare_dram_parameter("tri", [128, 128], dt, isOutput=False)
    cor_d = nc.declare_dram_parameter("cor", [128, 128], dt, isOutput=False)
    mask_d = nc.declare_dram_parameter("masklay", [128, LB * NK], dt, isOutput=False)
    alpha_d = nc.declare_dram_parameter("alphalay", [128, LB * NK], dt, isOutput=False)
    out_d = nc.declare_dram_parameter("out", [LB * NK, 128], dt, isOutput=True)

    TANH = mybir.ActivationFunctionType.Tanh
    SIG = mybir.ActivationFunctionType.Sigmoid
    IDENT = mybir.ActivationFunctionType.Identity
    MULT = mybir.AluOpType.mult
    ADD = mybir.AluOpType.add

    with tile.TileContext(nc) as tc:
        with (
            tc.tile_pool(name="const", bufs=1) as cpool,
            tc.tile_pool(name="encp", bufs=2) as encp,
            tc.tile_pool(name="lsp", bufs=2) as lsp,
            tc.tile_pool(name="xp", bufs=3) as xp,
            tc.tile_pool(name="xvp", bufs=2) as xvp,
            tc.tile_pool(name="tailp", bufs=2) as tailp,
            tc.tile_pool(name="zps", bufs=4, space="PSUM") as zps,
            tc.tile_pool(name="sps", bufs=2, space="PSUM") as sps,
        ):
            # ---- constants into SBUF ----
            eye_sb = cpool.tile([128, 128], dt, tag="eye")
            nc.sync.dma_start(out=eye_sb[:], in_=eye_d[:])
            ones_sb = cpool.tile([128, 128], dt, tag="ones")
            nc.sync.dma_start(out=ones_sb[:], in_=ones_d[:])
            tri_sb = cpool.tile([128, 128], dt, tag="tri")
            nc.sync.dma_start(out=tri_sb[:], in_=tri_d[:])
            cor_sb = cpool.tile([128, 128], dt, tag="cor")
            nc.sync.dma_start(out=cor_sb[:], in_=cor_d[:])
            mask_sb = cpool.tile([128, LB * NK], dt, tag="mask")
            nc.sync.dma_start(out=mask_sb[:], in_=mask_d[:])
            alpha_sb = cpool.tile([128, LB * NK], dt, tag="alpha")
            nc.sync.dma_start(out=alpha_sb[:], in_=alpha_d[:])

            # v broadcast to all 128 partitions (partition-step-0 DMA)
            v_sb = cpool.tile([128, A], dt, tag="vbc")
            va = vw_d[:]
            v_bcast = bass.AP(tensor=va.tensor, offset=va.offset,
                              ap=[[0, 128]] + [list(p) for p in va.ap])
            nc.sync.dma_start(out=v_sb[:], in_=v_bcast)

            # pq matmul operands
            qt_sb = cpool.tile([128, KP // 128, LB], dt, tag="qt")
            nc.sync.dma_start(out=qt_sb[:],
                              in_=qt_d.rearrange("(c p) n -> p c n", p=128))
            qw_sb = cpool.tile([128, KP // 128, A], dt, tag="qw")
            nc.sync.dma_start(out=qw_sb[:],
                              in_=qw_d.rearrange("(c p) a -> p c a", p=128))

            # per-batch rhs [63, A]: rows 0..61 folded conv+proj weight,
            # row 62 = pq[b] + W_b + L_b (filled after the pq matmul)
            rhs_sb = [cpool.tile([KC + 1, A], dt, name=f"rhs{b}", tag=f"rhs{b}")
                      for b in range(LB)]
            for b in range(LB):
                nc.sync.dma_start(out=rhs_sb[b][0:KC, :], in_=mcomb_d[:])

            u_sb = cpool.tile([128, LB * NK], dt, tag="u")
            eps_sb = cpool.tile([128, 1], dt, tag="eps")
            nc.vector.memset(eps_sb[:], 1e-7)

            for rep in range(repeats):
                # ---- processed query: pq = q @ W^T + (W_b + L_b) ----
                pq_ps = sps.tile([LB, A], dt, tag="tailps")
                for i in range(KP // 128):
                    nc.tensor.matmul(pq_ps[:], qt_sb[:, i, :], qw_sb[:, i, :],
                                     start=(i == 0), stop=(i == KP // 128 - 1))
                pq_sb = tailp.tile([LB, A], dt, tag="pqsb")
                nc.scalar.copy(out=pq_sb[:], in_=pq_ps[:])
                for b in range(LB):
                    nc.sync.dma_start(out=rhs_sb[b][KC:KC + 1, :],
                                      in_=pq_sb[b:b + 1, :])

                # ---- main loop: z = ploc+pq+enc ; x = tanh(z) ; u = x.v ----
                for b in range(LB):
                    enc_sb = encp.tile([128, NK, A], dt, tag="enc")
                    nc.sync.dma_start(
                        out=enc_sb[:],
                        in_=enc_d[b].rearrange("(k p) a -> p k a", p=128))
                    ls_sb = lsp.tile([KC + 1, T], dt, tag="ls")
                    nc.sync.dma_start(out=ls_sb[:], in_=ls_d[b])
                    for k in range(NK):
                        z_ps = zps.tile([128, A], dt, tag="z")
                        nc.tensor.matmul(z_ps[:], ls_sb[:, k * 128:(k + 1) * 128],
                                         rhs_sb[b][:], start=True, stop=False)
                        nc.tensor.matmul(z_ps[:], eye_sb[:], enc_sb[:, k, :],
                                         start=False, stop=True)
                        x_sb = xp.tile([128, A], dt, tag="x")
                        nc.scalar.activation(out=x_sb[:], in_=z_ps[:], func=TANH)
                        xv_sb = xvp.tile([128, A], dt, tag="xv")
                        col = k * LB + b
                        nc.vector.scalar_tensor_tensor(
                            out=xv_sb[:], in0=x_sb[:], scalar=1.0, in1=v_sb[:],
                            op0=MULT, op1=MULT,
                            accum_out=u_sb[:, col:col + 1])

                # ---- tail in [t', (k,b)] layout ----
                s_sb = tailp.tile([128, LB * NK], dt, tag="s")
                nc.scalar.activation(out=s_sb[:], in_=u_sb[:], func=SIG)
                att_sb = tailp.tile([128, LB * NK], dt, tag="att")
                nc.vector.tensor_mul(att_sb[:], s_sb[:], mask_sb[:])

                # w = alpha + shift1(alpha) + shift2(alpha) via band matmuls
                w_ps = sps.tile([128, LB * NK], dt, tag="tailps")
                nc.tensor.matmul(w_ps[:], tri_sb[:], alpha_sb[:],
                                 start=True, stop=False)
                nc.tensor.matmul(w_ps[:, LB:], cor_sb[:], alpha_sb[:, :-LB],
                                 start=False, stop=True, skip_group_check=True)
                w_sb = tailp.tile([128, LB * NK], dt, tag="w")
                nc.scalar.activation(out=w_sb[:], in_=w_ps[:], func=IDENT,
                                     bias=eps_sb[:], scale=1.0)

                na_sb = tailp.tile([128, LB * NK], dt, tag="na")
                nc.vector.tensor_mul(na_sb[:], att_sb[:], w_sb[:])

                # per-batch normalizer: colsum then sum over the k-groups
                cs_ps = sps.tile([1, LB * NK], dt, tag="tailps")
                nc.tensor.matmul(cs_ps[:], ones_sb[:, 0:1], na_sb[:],
                                 start=True, stop=True)
                z_sb = tailp.tile([1, LB], dt, tag="zsum")
                nc.vector.tensor_reduce(
                    out=z_sb[:], in_=cs_ps.rearrange("p (k b) -> p b k", b=LB),
                    axis=mybir.AxisListType.X, op=ADD)
                rz_sb = tailp.tile([1, LB], dt, tag="rz")
                nc.vector.reciprocal(out=rz_sb[:], in_=z_sb[:])
                rz64_sb = tailp.tile([1, LB * NK], dt, tag="rz64")
                rza = rz_sb[:]
                rz_b = bass.AP(tensor=rza.tensor, offset=rza.offset,
                               ap=[list(rza.ap[0]), [0, NK], list(rza.ap[1])])
                nc.vector.tensor_copy(
                    out=rz64_sb.rearrange("p (k b) -> p k b", b=LB), in_=rz_b)
                rb_ps = sps.tile([128, LB * NK], dt, tag="tailps")
                nc.tensor.matmul(rb_ps[:], ones_sb[0:1, :], rz64_sb[:],
                                 start=True, stop=True)
                nan_sb = tailp.tile([128, LB * NK], dt, tag="nan")
                nc.vector.tensor_mul(nan_sb[:], na_sb[:], rb_ps[:])

                # transpose to [(k b), t'] and store
                ot_ps = sps.tile([LB * NK, 128], dt, tag="tailps")
                nc.tensor.transpose(ot_ps[:], nan_sb[:], eye_sb[:])
                ot_sb = tailp.tile([LB * NK, 128], dt, tag="otsb")
                nc.scalar.copy(out=ot_sb[:], in_=ot_ps[:])
                nc.sync.dma_start(out=out_d[:], in_=ot_sb[:])

    _split_sync_waits(nc)
    return nc


def prep_inputs(inputs: dict) -> list[dict]:
    """Full inputs -> per-core in_maps (host layout prep only)."""
    enc = np.asarray(inputs["encoder_seq_proj"], np.float32)
    query = np.asarray(inputs["query"], np.float32)
    cum = np.asarray(inputs["cumulative"], np.float32)
    att = np.asarray(inputs["attention"], np.float32)
    alpha = np.asarray(inputs["alpha"], np.float32)
    conv_w = np.asarray(inputs["conv_w"], np.float32)
    L_w = np.asarray(inputs["L_w"], np.float32)
    L_b = np.asarray(inputs["L_b"], np.float32)
    W_w = np.asarray(inputs["W_w"], np.float32)
    W_b = np.asarray(inputs["W_b"], np.float32)
    v_w = np.asarray(inputs["v_w"], np.float32)
    phone_len = np.asarray(inputs["phone_len"], np.int64)

    # folded conv+projection weight: M[c*31+k, a] = sum_f conv_w[f,c,k]*L_w[a,f]
    mcomb = np.einsum("fck,af->cka", conv_w, L_w).reshape(KC, A)
    mcomb = np.ascontiguousarray(mcomb, np.float32)

    qw = np.zeros((KP, A), np.float32)
    qw[:A] = W_w.T
    qw[A] = W_b + L_b

    eye = np.eye(128, dtype=np.float32)
    ones = np.ones((128, 128), np.float32)
    # tri[s,t'] = 1 for t'-2 <= s <= t'  (alpha + shift1 + shift2, in-block)
    idx = np.arange(128)
    dif = idx[None, :] - idx[:, None]          # t' - s
    tri = ((dif >= 0) & (dif <= 2)).astype(np.float32)
    # cor[s,t']: cross-block corner terms from the previous 128-block
    cor = np.zeros((128, 128), np.float32)
    cor[126, 0] = 1.0
    cor[127, 0] = 1.0
    cor[127, 1] = 1.0

    mask = (np.arange(T)[None, :] < phone_len[:, None]).astype(np.float32)

    def lay(arr):  # [8,1024] -> [128, 64] with col = k*8 + b
        return np.ascontiguousarray(
            arr.reshape(LB, NK, 128).transpose(2, 1, 0).reshape(128, LB * NK))

    in_maps = []
    for c in range(NCORES):
        sl = slice(c * LB, (c + 1) * LB)
        cum_c, att_c = cum[sl], att[sl]
        ls = np.zeros((LB, KC + 1, T), np.float32)
        padc = np.zeros((LB, T + 2 * PAD), np.float32)
        pada = np.zeros((LB, T + 2 * PAD), np.float32)
        padc[:, PAD:PAD + T] = cum_c
        pada[:, PAD:PAD + T] = att_c
        for k in range(KW):
            ls[:, k, :] = padc[:, k:k + T]
            ls[:, KW + k, :] = pada[:, k:k + T]
        ls[:, KC, :] = 1.0

        qt = np.zeros((KP, LB), np.float32)
        qt[:A] = query[sl].T
        qt[A] = 1.0

        in_maps.append({
            "enc": np.ascontiguousarray(enc[sl]),
            "ls": ls,
            "qt": qt,
            "qw": qw,
            "mcomb": mcomb,
            "vw": np.ascontiguousarray(v_w[0]),
            "eye": eye,
            "ones": ones,
            "tri": tri,
            "cor": cor,
            "masklay": lay(mask[sl]),
            "alphalay": lay(alpha[sl]),
        })
    return in_maps


def assemble_output(results: list[dict]) -> np.ndarray:
    out = np.empty((B, 1, T), np.float32)
    for c in range(NCORES):
        oc = results[c]["out"]                      # [(k b), 128]
        oc = oc.reshape(NK, LB, 128).transpose(1, 0, 2).reshape(LB, T)
        out[c * LB:(c + 1) * LB, 0, :] = oc
    return out


_CACHED_NC = None


def kernel(**inputs) -> np.ndarray:
    global _CACHED_NC
    if _CACHED_NC is None:
        _CACHED_NC = build_program(repeats=1)
    in_maps = prep_inputs(inputs)
    res = run_bass_kernel_spmd(_CACHED_NC, in_maps, list(range(NCORES)))
    return assemble_output(res.results)



# revision 6
# speedup vs baseline: 410.6966x; 410.6966x over previous
"""Trainium2 Bass kernel for the forward-attention LSA step (nn_LSA_43404939494068).

Device computes only u[b,t] = sum_a v[a]*tanh(pq[b,a] + enc[b,t,a] + ploc[b,t,a]);
the query projection, sigmoid, masking, forward-attention recursion and
normalization run on the host in float64 (none of them touch the [B,T,A]
tensor, and the sigmoid-normalization of the reference cancels in the final
alpha normalization).

Sharding: data-parallel over batch, 8 batches per core (spec hint).

Per-core device program (pipelined over the 8 local batches):
  - enc[b] DMA'd contiguously: SBUF E[p, x*512+a] = enc[b, 8p+x, a]
    (t = 8p + x; 16KB per-partition descriptors)
  - the 31-tap conv over [cumulative; attention] and the L-projection fold
    into ONE K=63 matmul per [128t',512a] tile: 62 shifted-loc rows (host
    im2col) + a ones row whose rhs row carries pq[b]+W_b+L_b
  - DVE adds z+enc from PSUM (enc never passes through the PE), ACT tanh
    in place per 4-bank half, DVE multiply-accumulate against broadcast v
  - u[128, 64] DMA'd out; host finishes in float64

Execution: a jit'd shard_map runner is cached per-process; enc is passed
zero-copy as the full [64,1024,512] array (per-core slices are contiguous).
Under BASS_TRACE the original run_bass_kernel_spmd path is used so NTFF
profiling hooks keep working.
"""

import os
import sys

import numpy as np

if "/opt/trn_rl_repo" not in sys.path:
    sys.path.insert(0, "/opt/trn_rl_repo")

import concourse.bass as bass
import concourse.tile as tile
from concourse import mybir

B, T, A = 64, 1024, 512
F, KW = 32, 31
PAD = (KW - 1) // 2
NCORES = 8
LB = B // NCORES          # 8 local batches per core
NX = 8                    # t = 8*p + x, x in [0,8)
KC = 62                   # conv contraction = 2 channels * 31 taps
K = KC + 1                # + ones row (pq + biases)
LST = LB * T
F32 = mybir.dt.float32

_MAX_WAITS = 1


def _split_sync_waits(nc):
    """walrus in this toolchain accepts at most one sync-wait per
    instruction; hoist excess waits onto NoOps inserted just before."""
    for fn in nc.m.functions:
        for blk in fn.blocks:
            new_list = []
            for inst in blk.instructions:
                si = inst.sync_info
                if si is not None and si.on_wait and len(si.on_wait) > _MAX_WAITS:
                    waits = list(si.on_wait)
                    extra, keep = waits[:-_MAX_WAITS], waits[-_MAX_WAITS:]
                    for i in range(0, len(extra), _MAX_WAITS):
                        nop = mybir.InstNoOp(
                            name=nc.get_next_instruction_name(),
                            sync_info=mybir.SyncInfo(
                                on_wait=extra[i:i + _MAX_WAITS], on_update=[]
                            ),
                            bass_nofuse=True,
                            engine=inst.engine,
                        )
                        nc.register_instruction(nop)
                        new_list.append(nop)
                    inst.sync_info = mybir.SyncInfo(
                        on_wait=keep, on_update=list(si.on_update)
                    )
                new_list.append(inst)
            blk.instructions[:] = new_list


def build_program(repeats: int = 1) -> bass.Bass:
    nc = bass.Bass()
    dt = F32

    enc_d = nc.declare_dram_parameter("enc", [LB, T, A], dt, isOutput=False)
    lsr_d = nc.declare_dram_parameter("lsr", [K, LB * (T + A)], dt, isOutput=False)
    vw_d = nc.declare_dram_parameter("vw", [A], dt, isOutput=False)
    out_d = nc.declare_dram_parameter("uout", [128, LB * NX], dt, isOutput=True)

    TANH = mybir.ActivationFunctionType.Tanh
    MULT = mybir.AluOpType.mult
    ADD = mybir.AluOpType.add

    with tile.TileContext(nc) as tc:
        with (
            tc.tile_pool(name="const", bufs=1) as cpool,
            tc.tile_pool(name="lsp", bufs=2) as lsp,
            tc.tile_pool(name="encp", bufs=3) as encp,
            tc.tile_pool(name="xsp", bufs=2) as xsp,
            tc.tile_pool(name="zps", bufs=2, space="PSUM") as zps,
        ):
            # v broadcast to all 128 partitions (partition-step-0 DMA)
            v_sb = cpool.tile([128, A], dt, tag="vbc")
            va = vw_d[:]
            v_bcast = bass.AP(tensor=va.tensor, offset=va.offset,
                              ap=[[0, 128]] + [list(p) for p in va.ap])
            nc.sync.dma_start(out=v_sb[:], in_=v_bcast)

            u_sb = cpool.tile([128, LB * NX], dt, tag="u")

            for rep in range(repeats):
                lsr_sb = lsp.tile([K, LB * (T + A)], dt, tag="lsr")
                nc.sync.dma_start(out=lsr_sb[:], in_=lsr_d[:])

                for b in range(LB):
                    enc_sb = encp.tile([128, NX * A], dt, tag="enc")
                    nc.scalar.dma_start(
                        out=enc_sb[:],
                        in_=enc_d[b].rearrange("(p x) a -> p (x a)", p=128))
                    xs_sb = xsp.tile([128, NX * A], dt, tag="xs")
                    for h in range(2):          # half-tiles: 4 PSUM banks each
                        z_ps = zps.tile([128, 4 * A], dt, tag="z")
                        for q in range(4):
                            x = 4 * h + q
                            nc.tensor.matmul(
                                z_ps[:, q * A:(q + 1) * A],
                                lsr_sb[:, b * T + x * 128:b * T + (x + 1) * 128],
                                lsr_sb[:, LST + b * A:LST + (b + 1) * A],
                                start=True, stop=True)
                        sl = slice(h * 4 * A, (h + 1) * 4 * A)
                        nc.vector.tensor_tensor(
                            out=xs_sb[:, sl], in0=z_ps[:], in1=enc_sb[:, sl],
                            op=ADD)
                        nc.scalar.activation(out=xs_sb[:, sl], in_=xs_sb[:, sl],
                                             func=TANH)
                        for x in range(4 * h, 4 * h + 4):
                            exa = xs_sb[:, x * A:(x + 1) * A]
                            nc.vector.scalar_tensor_tensor(
                                out=exa, in0=exa, scalar=1.0, in1=v_sb[:],
                                op0=MULT, op1=MULT,
                                accum_out=u_sb[:, b * NX + x:b * NX + x + 1])

                nc.sync.dma_start(out=out_d[:], in_=u_sb[:])

    _split_sync_waits(nc)
    return nc


def prep_global(inputs: dict):
    """Full inputs -> global (concatenated-over-cores) device arrays.
    enc passes through zero-copy; lsr/vw are built per core and stacked."""
    enc = np.ascontiguousarray(np.asarray(inputs["encoder_seq_proj"], np.float32))
    query = np.asarray(inputs["query"], np.float32)
    cum = np.asarray(inputs["cumulative"], np.float32)
    att = np.asarray(inputs["attention"], np.float32)
    conv_w = np.asarray(inputs["conv_w"], np.float32)
    L_w = np.asarray(inputs["L_w"], np.float32)
    L_b = np.asarray(inputs["L_b"], np.float32)
    W_w = np.asarray(inputs["W_w"], np.float32)
    W_b = np.asarray(inputs["W_b"], np.float32)
    v_w = np.asarray(inputs["v_w"], np.float32)

    # folded conv+projection weight: M[c*31+k, a] = sum_f conv_w[f,c,k]*L_w[a,f]
    mcomb = np.einsum("fck,af->cka", conv_w, L_w).reshape(KC, A)
    pqf = (query @ W_w.T + W_b + L_b).astype(np.float32)      # [B, A]

    # ls rows: j=0..30 cumulative shifts, j=31..61 attention shifts, j=62 ones;
    # column layout per core: (b, x, p) with t = 8p + x
    ls = np.zeros((K, B, T), np.float32)
    padc = np.zeros((B, T + 2 * PAD), np.float32)
    pada = np.zeros((B, T + 2 * PAD), np.float32)
    padc[:, PAD:PAD + T] = cum
    pada[:, PAD:PAD + T] = att
    for k in range(KW):
        ls[k] = padc[:, k:k + T]
        ls[KW + k] = pada[:, k:k + T]
    ls[KC] = 1.0
    # [K, B, T] -> t = 8p+x -> [K, B, 8x, 128p]
    ls_dev = np.ascontiguousarray(
        ls.reshape(K, B, 128, NX).transpose(0, 1, 3, 2)).reshape(K, B, T)

    rhs = np.empty((K, B, A), np.float32)
    rhs[:KC] = mcomb[:, None, :]
    rhs[KC] = pqf

    # global lsr: per core c the block [K, LB*(T+A)]; stacked on axis 0
    lsr = np.empty((NCORES * K, LB * (T + A)), np.float32)
    for c in range(NCORES):
        sl = slice(c * LB, (c + 1) * LB)
        lsr[c * K:(c + 1) * K, :LST] = ls_dev[:, sl].reshape(K, LST)
        lsr[c * K:(c + 1) * K, LST:] = rhs[:, sl].reshape(K, LB * A)

    vw = np.tile(np.ascontiguousarray(v_w[0]), NCORES)        # [8*A]
    return {"enc": enc, "lsr": lsr, "vw": vw}


def prep_inputs(inputs: dict) -> list[dict]:
    """Per-core in_maps view (for run_bass_kernel_spmd / tracing path)."""
    g = prep_global(inputs)
    return [{
        "enc": g["enc"][c * LB:(c + 1) * LB],
        "lsr": g["lsr"][c * K:(c + 1) * K],
        "vw": g["vw"][c * A:(c + 1) * A],
    } for c in range(NCORES)]


def postprocess(inputs: dict, uouts: list[np.ndarray]) -> np.ndarray:
    """u -> sigmoid -> mask -> forward-attention recursion -> normalize.
    All on the host in float64; touches only [B,T]-sized tensors."""
    alpha = np.asarray(inputs["alpha"], np.float64)
    phone_len = np.asarray(inputs["phone_len"])
    mask = (np.arange(T)[None, :] < phone_len[:, None]).astype(np.float64)

    u = np.empty((B, T), np.float64)
    for c in range(NCORES):
        uo = np.asarray(uouts[c], np.float64)       # [128, LB*NX]
        for b in range(LB):
            # col = b*8 + x, row = p, t = 8p + x
            u[c * LB + b] = uo[:, b * NX:(b + 1) * NX].reshape(T)

    s = 1.0 / (1.0 + np.exp(-u))
    shift = np.zeros_like(alpha)
    shift[:, 1:] = alpha[:, :-1]
    dual = np.zeros_like(alpha)
    dual[:, 2:] = alpha[:, :-2]
    na = (shift + dual + alpha + 1e-7) * s * mask
    na = na / na.sum(axis=-1, keepdims=True)
    return na[:, None, :].astype(np.float32)


_CACHED = {}


def _make_runner(nc):
    """jit'd shard_map runner over 8 cores, built once per process."""
    import jax
    from jax.sharding import Mesh, PartitionSpec
    from jax.experimental.shard_map import shard_map
    from concourse.bass2jax import (_bass_exec_p, install_neuronx_cc_hook,
                                    partition_id_tensor)

    install_neuronx_cc_hook()
    partition_name = nc.partition_id_tensor.name if nc.partition_id_tensor else None

    in_names, out_names, out_avals = [], [], []
    for alloc in nc.m.functions[0].allocations:
        if not isinstance(alloc, mybir.MemoryLocationSet):
            continue
        name = alloc.memorylocations[0].name
        if alloc.kind == "ExternalInput":
            if name != partition_name:
                in_names.append(name)
        elif alloc.kind == "ExternalOutput":
            out_names.append(name)
            out_avals.append(jax.core.ShapedArray(
                tuple(alloc.tensor_shape), mybir.dt.np(alloc.dtype)))
    all_in = list(in_names) + list(out_names)
    if partition_name is not None:
        all_in.append(partition_name)

    def _body(*args):
        operands = list(args)
        if partition_name is not None:
            operands.append(partition_id_tensor())
        return tuple(_bass_exec_p.bind(
            *operands,
            out_avals=tuple(out_avals),
            in_names=tuple(all_in),
            out_names=tuple(out_names),
            lowering_input_output_aliases=(),
            sim_require_finite=True,
            sim_require_nnan=True,
            nc=nc,
        ))

    devices = jax.devices()[:NCORES]
    mesh = Mesh(np.asarray(devices), ("core",))
    nin = len(in_names) + len(out_names)
    fn = jax.jit(
        shard_map(_body, mesh=mesh,
                  in_specs=(PartitionSpec("core"),) * nin,
                  out_specs=(PartitionSpec("core"),) * len(out_names),
                  check_rep=False),
        keep_unused=True,
    )
    zero_shapes = [(NCORES * av.shape[0], *av.shape[1:]) for av in out_avals]
    zero_dtypes = [av.dtype for av in out_avals]

    def run(globals_map):
        args = [globals_map[nm] for nm in in_names]
        args += [np.zeros(s, d) for s, d in zip(zero_shapes, zero_dtypes)]
        outs = fn(*args)
        return {nm: np.asarray(o) for nm, o in zip(out_names, outs)}

    return run


def kernel(**inputs) -> np.ndarray:
    if os.environ.get("BASS_TRACE"):
        # tracing path: per-core in_maps through run_bass_kernel_spmd so the
        # NTFF profile hook sees the execution
        from concourse.bass_utils import run_bass_kernel_spmd
        if "nc" not in _CACHED:
            _CACHED["nc"] = build_program(repeats=1)
        in_maps = prep_inputs(inputs)
        res = run_bass_kernel_spmd(_CACHED["nc"], in_maps, list(range(NCORES)))
        return postprocess(inputs, [r["uout"] for r in res.results])

    if "run" not in _CACHED:
        _CACHED["nc"] = build_program(repeats=1)
        _CACHED["run"] = _make_runner(_CACHED["nc"])
    g = prep_global(inputs)
    out = _CACHED["run"](g)
    uo = out["uout"]                                  # [8*128, LB*NX]
    uouts = [uo[c * 128:(c + 1) * 128] for c in range(NCORES)]
    return postprocess(inputs, uouts)
